# revision 47
# baseline (speedup 1.0000x reference)
"""Cosformer (linear) attention kernel for 8 TRN2 NeuronCores.

Full (unsharded) inputs in, full output out.  Sharding: 8 cores =
4 batches x 2 head-halves.  Core c handles batch b = c//2 and heads
[hh*8, hh*8+8) where hh = c%2, i.e. embed cols [hh*512, (hh+1)*512).

Per-core math (all shapes per core):
  xT = x[:, b, :].T                        (E=1024, L=2048)  for q/k/v
  k  = relu(x_k @ Wk_s.T + bk_s)           [L, 512]  (L on partitions)
  v  =      x_v @ Wv_s.T + bv_s            [L, 512]
  qT = relu(Wq_s @ x_q.T + bq_s)           [512, L]  (head dims on partitions)
  per head h (64 dims):
    k_ = [k*sin | k*cos]                   [L, 128]
    KV_aug = k_.T @ [v | 1]                [128, 65]   (col 64 = sum_l k_)
  attention (no q duplication): with qts = qT*sin_l, qtc = qT*cos_l
  (per-L-column scaling on DVE, one pass per quarter), and block-diagonal
  KV tensors kvs_bd/kvc_bd [128, oc, 130] pairing head 2oc (rows 0:64,
  cols 0:65) with head 2oc+1 (rows 64:128, cols 65:130):
    po = qts_chunk.T @ kvs_bd[oc] + qtc_chunk.T @ kvc_bd[oc]   (PSUM acc)
    po[:, t*65 : t*65+65] = o_aug of head 2oc+t
    o = o_aug[:, :64] / max(o_aug[:, 64], EPS)
"""

import math
from contextlib import ExitStack

import numpy as np
import ml_dtypes

BF = ml_dtypes.bfloat16

import concourse.bass as bass
import concourse.bacc as bacc_mod
import concourse.mybir as mybir
from concourse.tile import TileContext
from concourse.bass_utils import run_bass_kernel_spmd

L = 2048            # sequence length
NB = 4              # batch
E = 1024            # embed dim
D = 64              # head dim
HC = 8              # heads per core
OC = HC * D         # 512 embed cols per core
P = 128
KC = E // P         # 8 contraction chunks over E
LC = L // P         # 16 L chunks of 128
NLC = L // 512      # 4 L chunks of 512
OCC = OC // P       # 4 q-proj output chunks
EPS = 1e-4

F32 = mybir.dt.float32
F32R = mybir.dt.float32r
BF16 = mybir.dt.bfloat16
AF = mybir.ActivationFunctionType

BD = 2 * (D + 1)    # block-diag kv free width (two heads' aug columns)


def build_nc(with_bias=True):
    nc = bacc_mod.Bacc()

    # x/w come in pre-swizzled by the host so every DMA slice is contiguous
    # per partition (big descriptors -> full DMA-queue bandwidth):
    #   x*: [P, NLC, KC, 512]  (group-major),  w*: [P, KC, OC]
    xq = nc.declare_dram_parameter("xq", [P, NLC * KC * 512], BF16,
                                   isOutput=False)
    xk = nc.declare_dram_parameter("xk", [P, NLC * KC * 512], BF16,
                                   isOutput=False)
    xv = nc.declare_dram_parameter("xv", [P, NLC * KC * 512], BF16,
                                   isOutput=False)
    wq = nc.declare_dram_parameter("wq", [P, KC * OC], BF16, isOutput=False)
    wk = nc.declare_dram_parameter("wk", [P, KC * OC], BF16, isOutput=False)
    wv = nc.declare_dram_parameter("wv", [P, KC * OC], BF16, isOutput=False)
    bqc = nc.declare_dram_parameter("bqc", [P, OCC], F32, isOutput=False)
    bkr = nc.declare_dram_parameter("bkr", [1, OC], BF16, isOutput=False)
    bvr = nc.declare_dram_parameter("bvr", [1, OC], BF16, isOutput=False)
    onesr = nc.declare_dram_parameter("onesr", [1, P], BF16, isOutput=False)
    scs = nc.declare_dram_parameter("scs", [P, L], BF16, isOutput=False)
    scc = nc.declare_dram_parameter("scc", [P, L], BF16, isOutput=False)
    # sin and cos per-lc columns packed in one tensor: one dma_start,
    # 128B descriptors instead of 2x64B
    sccol = nc.declare_dram_parameter("sccol", [P, 2 * LC], F32,
                                      isOutput=False)
    outd = nc.declare_dram_parameter("out", [L, OC], BF16, isOutput=True)

    xq_r = xq.rearrange("p (g kc l) -> p g kc l", g=NLC, kc=KC)
    xk_r = xk.rearrange("p (g kc l) -> p g kc l", g=NLC, kc=KC)
    xv_r = xv.rearrange("p (g kc l) -> p g kc l", g=NLC, kc=KC)
    wk_r = wk.rearrange("p (kc o) -> p kc o", kc=KC)
    wv_r = wv.rearrange("p (kc o) -> p kc o", kc=KC)
    wq_r = wq.rearrange("p (kc o) -> p kc o", kc=KC)
    out_r = outd.rearrange("(lc p) o -> lc p o", p=P)

    with TileContext(nc) as tc, ExitStack() as ctx:
        const = ctx.enter_context(tc.tile_pool(name="const", bufs=1))
        persist = ctx.enter_context(tc.tile_pool(name="persist", bufs=1))
        xqp = ctx.enter_context(tc.tile_pool(name="xqp", bufs=1))
        # q-proj PSUM pool lives at the outer scope: quarter 0 is emitted
        # inside phase 1 (its matmuls hide the kv_acc(15) ksc/va latency).
        # Phase-1 PSUM: projp 4 + kvp 2 + pqp 2 = 8 banks exactly.
        pqp = ctx.enter_context(tc.tile_pool(name="pqp", bufs=2,
                                             space="PSUM"))

        wk_t = const.tile([P, KC, OC], BF16)
        wv_t = const.tile([P, KC, OC], BF16)
        wq_t = const.tile([P, KC, OC], BF16)
        bq_t = const.tile([P, OCC], F32)
        bk_t = const.tile([1, OC], BF16)
        bv_t = const.tile([1, OC], BF16)
        sc_col = const.tile([P, 2, LC], F32)
        ones_t = const.tile([1, P], BF16)

        scs_t = persist.tile([P, L], BF16)   # sin_l broadcast to all rows
        scc_t = persist.tile([P, L], BF16)   # cos_l broadcast to all rows
        # per-quarter tiles (separate tiles keep dep-tracking fine-grained).
        # qt_sb holds raw relu(q); qts_sb = qt*sin; qt_sb is then overwritten
        # in place with qt*cos.
        qt_sb = [persist.tile([P, OCC, 512], BF16, name=f"qt{n}")
                 for n in range(NLC)]
        qts_sb = [persist.tile([P, OCC, 512], BF16, name=f"qts{n}")
                  for n in range(NLC)]
        kv_sb = persist.tile([P, HC, D + 2], BF16)   # per-head KV_aug
        kvs_bd = persist.tile([P, OCC, BD], BF16)    # block-diag sin-KV
        kvc_bd = persist.tile([P, OCC, BD], BF16)    # block-diag cos-KV

        # ---------------- phase 1: k/v projections + KV accumulation -------
        with ExitStack() as p1:
            xkp = p1.enter_context(tc.tile_pool(name="xkp", bufs=1))
            xvp = p1.enter_context(tc.tile_pool(name="xvp", bufs=1))
            warmp = p1.enter_context(tc.tile_pool(name="warmp", bufs=1))
            kscp = p1.enter_context(tc.tile_pool(name="kscp", bufs=6))
            vap = p1.enter_context(tc.tile_pool(name="vap", bufs=3))
            projp = p1.enter_context(tc.tile_pool(name="projp", bufs=4,
                                                  space="PSUM"))
            kvp = p1.enter_context(tc.tile_pool(name="kvp", bufs=1,
                                                space="PSUM"))

            kv_ps = [
                kvp.tile([P, 4, D + 2], F32, name="kv_ps0"),
                kvp.tile([P, 4, D + 2], F32, name="kv_ps1"),
            ]

            # HAM warm-up: keep PE busy during the initial DMA ramp so the
            # clock gate opens before the first real matmuls.  Results are
            # discarded (kv_ps0 is reset by the real chain's start=True).
            warm_t = warmp.tile([P, 2 * P], BF16, name="warm_t")
            nc.vector.memset(warm_t[:, :], 0.0)
            for w in range(44):
                nc.tensor.matmul(kv_ps[0][:, 0:2, :], warm_t[:, 0:P],
                                 warm_t[:, 0:2 * (D + 2)],
                                 start=True, stop=True)

            # ---- intro (lc 0-3): kc-major so compute starts as soon as the
            # first (wk chunk, xk chunk) pair lands.  DMA issue order IS the
            # HWDGE service order per queue: interleave per-kc pairs.
            xk_t0 = xkp.tile([P, KC, 512], BF16, tag="xk_g0", name="xk_t0")
            xv_t0 = xvp.tile([P, KC, 512], BF16, tag="xv_g0", name="xv_t0")
            # DMA engine slots cost ~constant time per DESCRIPTOR (one per
            # partition row), so per-partition-contiguous size is king:
            # 8KB descriptors move ~3x the bytes/slot of 1KB ones.  Issue
            # each intro tensor as a small head chunk (first kc, so the
            # first matmul fires ASAP) + one big tail chunk (7 contiguous
            # kc = 7KB descriptors).  Weights ride the sync ring, x the
            # scalar ring, so both streams ramp together.
            intro_chunks = [(0, 1), (1, 4), (4, 8)]
            for c0, c1 in intro_chunks:
                nc.sync.dma_start(out=wk_t[:, c0:c1, :], in_=wk_r[:, c0:c1, :])
                nc.scalar.dma_start(out=xk_t0[:, c0:c1, :],
                                    in_=xk_r[:, 0, c0:c1, :])
            nc.sync.dma_start(out=sc_col, in_=sccol[:, :])
            if with_bias:
                nc.sync.dma_start(out=bk_t, in_=bkr[:, :])
                nc.sync.dma_start(out=bv_t, in_=bvr[:, :])
                nc.sync.dma_start(out=ones_t, in_=onesr[:, :])
            for c0, c1 in intro_chunks:
                nc.sync.dma_start(out=wv_t[:, c0:c1, :], in_=wv_r[:, c0:c1, :])
                nc.scalar.dma_start(out=xv_t0[:, c0:c1, :],
                                    in_=xv_r[:, 0, c0:c1, :])
            # group-1 prefetch (lands while the intro computes); groups 2/3
            # are issued inside the steady loop (ping-pong buffers)
            xk_ts = {0: xk_t0}
            xv_ts = {0: xv_t0}

            def prefetch_x(g):
                xk_tg = xkp.tile([P, KC, 512], BF16, tag=f"xk_g{g % 2}",
                                 name="xk_tg")
                xv_tg = xvp.tile([P, KC, 512], BF16, tag=f"xv_g{g % 2}",
                                 name="xv_tg")
                nc.scalar.dma_start(out=xk_tg, in_=xk_r[:, g])
                nc.sync.dma_start(out=xv_tg, in_=xv_r[:, g])
                xk_ts[g] = xk_tg
                xv_ts[g] = xv_tg

            prefetch_x(1)

            ksc_ts = {}
            va_ts = {}

            def proj_block(tag, x_t, w_t, b_t):
                """kc-major 4-lc projection block; returns 4 psum tiles."""
                p_ts = [projp.tile([P, OC], F32, tag="proj", name=f"p_{tag}{i}")
                        for i in range(4)]
                for kc in range(KC):
                    for i in range(4):
                        nc.tensor.matmul(p_ts[i][:, :],
                                         x_t[:, kc, i * P:(i + 1) * P],
                                         w_t[:, kc, :],
                                         start=(kc == 0),
                                         stop=(not with_bias and kc == KC - 1))
                if with_bias:
                    for i in range(4):
                        nc.tensor.matmul(p_ts[i][:, :], ones_t[:, :], b_t[:, :],
                                         start=False, stop=True)
                return p_ts

            def make_ksc(lc, pk_t):
                # k_sc[p,h,0,:] = relu(k)*sin_l ; k_sc[p,h,1,:] = relu(k)*cos_l
                # (sin/cos >= 0 on (0, pi/2], so relu(k*s) == relu(k)*s)
                ksc_t = kscp.tile([P, HC, 2, D], BF16, tag="ksc", name="ksc_t")
                pk_v = pk_t.rearrange("p (h d) -> p h d", d=D)
                nc.scalar.activation(ksc_t[:, :, 0, :], pk_v, AF.Relu,
                                     scale=sc_col[:, 0, lc:lc + 1])
                nc.scalar.activation(ksc_t[:, :, 1, :], pk_v, AF.Relu,
                                     scale=sc_col[:, 1, lc:lc + 1])
                ksc_ts[lc] = ksc_t

            def make_va(lc, pv_t):
                va_t = vap.tile([P, HC, D + 2], BF16, tag="va", name="va_t")
                pv_v = pv_t.rearrange("p (h d) -> p h d", d=D)
                nc.scalar.activation(va_t[:, :, D:D + 2], pv_v[:, :, 0:2],
                                     AF.Copy, bias=1.0, scale=0.0)
                nc.vector.tensor_copy(va_t[:, :, 0:D], pv_v)
                va_ts[lc] = va_t

            def kv_acc(lc):
                # KV_aug accumulation: 4 heads share one PSUM bank; only the
                # very first matmul into each bank uses start=True (clears
                # has_written bank-wide), everything else start=False so the
                # per-element has_written bits do the right thing.
                ksc_t, va_t = ksc_ts.pop(lc), va_ts.pop(lc)
                for h in range(HC):
                    nc.tensor.matmul(
                        kv_ps[h // 4][:, h % 4, :],
                        ksc_t[:, h, :, :],
                        va_t[:, h, :],
                        start=(lc == 0 and h % 4 == 0),
                        stop=(lc == LC - 1 and h % 4 == 3),
                    )

            pk_ts = proj_block("k", xk_t0, wk_t, bk_t)
            for lc in range(4):
                make_ksc(lc, pk_ts[lc])
            pv_ts = proj_block("v", xv_t0, wv_t, bv_t)
            for lc in range(4):
                make_va(lc, pv_ts[lc])
                kv_acc(lc)

            # ---- steady (lc 4-15): lc-major
            xq_ts = []
            for lc in range(4, LC):
                g = lc // 4
                if lc == 4:
                    prefetch_x(2)
                elif lc == 8:
                    prefetch_x(3)
                    # q-phase loads sit AFTER prefetch_x(3) in the ring
                    # FIFOs, so they stream through the otherwise-idle back
                    # half of phase 1 instead of competing with the k/v
                    # steady prefetches.
                    nc.scalar.dma_start(out=wq_t, in_=wq_r)
                    for n in range(2):
                        xq_t = xqp.tile([P, KC, 512], BF16, tag=f"xq{n}",
                                        name="xq_t")
                        eng = nc.sync if n % 2 == 0 else nc.scalar
                        eng.dma_start(out=xq_t, in_=xq_r[:, n])
                        xq_ts.append(xq_t)
                    nc.sync.dma_start(out=scs_t, in_=scs[:, :])
                    nc.scalar.dma_start(out=scc_t, in_=scc[:, :])
                elif lc == 10:
                    for n in range(2, NLC):
                        xq_t = xqp.tile([P, KC, 512], BF16, tag=f"xq{n}",
                                        name="xq_t")
                        eng = nc.sync if n % 2 == 0 else nc.scalar
                        eng.dma_start(out=xq_t, in_=xq_r[:, n])
                        xq_ts.append(xq_t)
                    nc.sync.dma_start(out=bq_t, in_=bqc[:, :])
                j4 = (lc % 4) * P
                xk_t = xk_ts[g][:, :, j4:j4 + P]
                xv_t = xv_ts[g][:, :, j4:j4 + P]

                pk_t = projp.tile([P, OC], F32, tag="proj", name="pk_t")
                for kc in range(KC):
                    nc.tensor.matmul(pk_t[:, :], xk_t[:, kc, :], wk_t[:, kc, :],
                                     start=(kc == 0),
                                     stop=(not with_bias and kc == KC - 1))
                if with_bias:
                    nc.tensor.matmul(pk_t[:, :], ones_t[:, :], bk_t[:, :],
                                     start=False, stop=True)
                make_ksc(lc, pk_t)
                # kv_acc lags one lc: lc-1's ksc/va (ACT+DVE latency off the
                # k-proj psum) completes under THIS lc's k matmuls, so the
                # PE never waits on the epilogue chain -- in particular not
                # at the phase-1 -> phase-2 boundary.
                if lc > 4:
                    kv_acc(lc - 1)

                pv_t = projp.tile([P, OC], F32, tag="proj", name="pv_t")
                for kc in range(KC):
                    nc.tensor.matmul(pv_t[:, :], xv_t[:, kc, :], wv_t[:, kc, :],
                                     start=(kc == 0),
                                     stop=(not with_bias and kc == KC - 1))
                if with_bias:
                    nc.tensor.matmul(pv_t[:, :], ones_t[:, :], bv_t[:, :],
                                     start=False, stop=True)
                make_va(lc, pv_t)

            def qproj_quarter(n, hook=None):
                # per-oc: matmuls, relu evict, then the sin/cos scaling muls
                # IMMEDIATELY (per-oc on DVE): each mul's relu dependency
                # resolves partway through the quarter, so the muls drain
                # during the quarter's own matmul window and never
                # head-of-line-block a later epilogue in the DVE FIFO.
                qt_n, qts_n = qt_sb[n], qts_sb[n]
                r = slice(n * 512, (n + 1) * 512)
                for oc in range(OCC):
                    pq_t = pqp.tile([P, 512], F32, tag="pq", name="pq_t")
                    for kc in range(KC):
                        nc.tensor.matmul(
                            pq_t[:, :],
                            wq_t[:, kc, oc * P:(oc + 1) * P],
                            xq_ts[n][:, kc, :],
                            start=(kc == 0), stop=(kc == KC - 1))
                    nc.scalar.activation(qt_n[:, oc, :], pq_t[:, :],
                                         AF.Relu, bias=bq_t[:, oc:oc + 1])
                    nc.vector.tensor_tensor(qts_n[:, oc, :], qt_n[:, oc, :],
                                            scs_t[:, r],
                                            mybir.AluOpType.mult)
                    nc.vector.tensor_tensor(qt_n[:, oc, :], qt_n[:, oc, :],
                                            scc_t[:, r],
                                            mybir.AluOpType.mult)
                    if oc == 0 and hook is not None:
                        hook()

            kv_acc(LC - 1)

            # evict KV accumulators to SBUF on ACT
            nc.scalar.activation(kv_sb[:, 0:4, :], kv_ps[0][:, :, :], AF.Copy)
            nc.scalar.activation(kv_sb[:, 4:8, :], kv_ps[1][:, :, :], AF.Copy)

        # ---- block-diagonal KV build (phase boundary; rings are idle) ----
        # kvs_bd[:, oc]: rows 0:64 cols 0:65 = KV_sin of head 2oc,
        #                rows 64:128 cols 65:130 = KV_sin of head 2oc+1.
        # kvc_bd likewise with the cos halves.  Off-blocks must be ZERO
        # (they are accumulated over by the paired matmul).
        nc.vector.memset(kvs_bd[:, :, :], 0.0)
        nc.vector.memset(kvc_bd[:, :, :], 0.0)
        kvv = kv_sb.rearrange("p (o t) m -> p o t m", t=2)
        DA = D + 1
        # partition-preserving halves on DVE
        nc.vector.tensor_copy(kvs_bd[0:D, :, 0:DA], kvv[0:D, :, 0, 0:DA])
        nc.vector.tensor_copy(kvc_bd[D:P, :, DA:BD], kvv[D:P, :, 1, 0:DA])
        # partition-crossing halves via SBUF->SBUF DMA
        nc.sync.dma_start(out=kvs_bd[D:P, :, DA:BD], in_=kvv[0:D, :, 1, 0:DA])
        nc.scalar.dma_start(out=kvc_bd[0:D, :, 0:DA], in_=kvv[D:P, :, 0, 0:DA])

        # ---------------- phase 2: q projection + attention ----------------
        with ExitStack() as p2:
            pop = p2.enter_context(tc.tile_pool(name="pop", bufs=3,
                                                space="PSUM"))
            osbp = p2.enter_context(tc.tile_pool(name="osbp", bufs=8))
            zp = p2.enter_context(tc.tile_pool(name="zp", bufs=8))

            def attn_lc(lc):
                # per oc-pair: two accumulating matmuls (sin- and cos-half)
                # into one PSUM region; col t*65+64 is head 2oc+t's
                # denominator.  po tiles span exactly 2 banks (2 oc each).
                # All 8 matmuls are emitted before the epilogues so the PE
                # stream never sits behind an epilogue dependency; one
                # half's final scale runs on ACT to balance DVE.
                n = lc // 4
                j4 = (lc % 4) * P
                o_t = osbp.tile([P, OC], BF16, tag="osb", name="o_t")
                ov = o_t.rearrange("p (o t d) -> p o t d", o=OCC, d=D)
                for half in range(2):
                    po_t = pop.tile([P, 2, 512], F32, tag="po", name="po_t")
                    for i in range(2):
                        oc = half * 2 + i
                        nc.tensor.matmul(po_t[:, i, 0:BD],
                                         qts_sb[n][:, oc, j4:j4 + P],
                                         kvs_bd[:, oc, :],
                                         start=True, stop=False)
                        nc.tensor.matmul(po_t[:, i, 0:BD],
                                         qt_sb[n][:, oc, j4:j4 + P],
                                         kvc_bd[:, oc, :],
                                         start=False, stop=True)
                    pov = po_t[:, :, 0:BD].rearrange(
                        "p i (t m) -> p i t m", t=2)
                    # the eps clip never binds: den is a sum of 2048
                    # nonnegative relu products, orders of magnitude > eps
                    zr_t = zp.tile([P, 2, 2], F32, tag="zr", name="zr_t")
                    nc.vector.reciprocal(zr_t[:, :, :], pov[:, :, :, D])
                    zb = zr_t[:, :, :].unsqueeze(3).broadcast_to(
                        (P, 2, 2, D))
                    nc.vector.tensor_tensor(ov[:, 2 * half:2 * half + 2],
                                            pov[:, :, :, 0:D], zb,
                                            mybir.AluOpType.mult)
                eng = nc.sync if lc % 2 == 0 else nc.scalar
                eng.dma_start(out=out_r[lc], in_=o_t[:, :])

            # software-pipelined: attention block b launches one quarter
            # after q-proj quarter b; its only dependencies are qts/qtc
            # (DVE, right after quarter b's relu evicts) and the kv_bd
            # tiles (built at the phase boundary).  No DMA on the critical
            # path, so only block 3 trails the last q-proj.
            qproj_quarter(0)
            qproj_quarter(1)
            for lc in range(0, 4):
                attn_lc(lc)
            qproj_quarter(2)
            for lc in range(4, 8):
                attn_lc(lc)
            qproj_quarter(3)
            for lc in range(8, 16):
                attn_lc(lc)

    nc.compile()
    return nc


_NC = {}


def _get_nc(with_bias=True):
    if with_bias not in _NC:
        _NC[with_bias] = build_nc(with_bias)
    return _NC[with_bias]


def _host_constants():
    idx = (math.pi / 2.0) * (np.arange(L, dtype=np.float64) + 1.0) / float(L)
    sinv = np.sin(idx).astype(np.float32)
    cosv = np.cos(idx).astype(np.float32)
    return {
        "scs": np.ascontiguousarray(
            np.broadcast_to(sinv[None, :], (P, L))).astype(BF),
        "scc": np.ascontiguousarray(
            np.broadcast_to(cosv[None, :], (P, L))).astype(BF),
        "sccol": np.ascontiguousarray(np.stack(
            [sinv.reshape(LC, P).T, cosv.reshape(LC, P).T],
            axis=1).reshape(P, 2 * LC)),
        "onesr": np.ones((1, P), BF),
    }


def _swizzle_x(xT):
    """[E, L] -> [P, NLC*KC*512]: x[p, g, kc, j] = xT[kc*P+p, g*512+j]."""
    v = xT.reshape(KC, P, NLC, 512).transpose(1, 2, 0, 3)
    return np.ascontiguousarray(v).reshape(P, NLC * KC * 512)


def _swizzle_w(wT):
    """[E, OC] -> [P, KC*OC]: w[p, kc, o] = wT[kc*P+p, o]."""
    v = wT.reshape(KC, P, OC).transpose(1, 0, 2)
    return np.ascontiguousarray(v).reshape(P, KC * OC)


def kernel(query, key, value, Wq, bq, Wk, bk, Wv, bv):
    query = np.asarray(query, np.float32)
    key = np.asarray(key, np.float32)
    value = np.asarray(value, np.float32)
    Wq = np.asarray(Wq, np.float32)
    Wk = np.asarray(Wk, np.float32)
    Wv = np.asarray(Wv, np.float32)
    bq = np.asarray(bq, np.float32)
    bk = np.asarray(bk, np.float32)
    bv = np.asarray(bv, np.float32)

    consts = _host_constants()
    in_maps = []
    for c in range(8):
        b, hh = divmod(c, 2)
        sl = slice(hh * OC, (hh + 1) * OC)
        in_maps.append({
            "xq": _swizzle_x(query[:, b, :].T).astype(BF),
            "xk": _swizzle_x(key[:, b, :].T).astype(BF),
            "xv": _swizzle_x(value[:, b, :].T).astype(BF),
            "wq": _swizzle_w(Wq[sl, :].T).astype(BF),
            "wk": _swizzle_w(Wk[sl, :].T).astype(BF),
            "wv": _swizzle_w(Wv[sl, :].T).astype(BF),
            "bqc": np.ascontiguousarray(bq[sl].reshape(OCC, P).T),
            "bkr": np.ascontiguousarray(bk[sl].reshape(1, OC)).astype(BF),
            "bvr": np.ascontiguousarray(bv[sl].reshape(1, OC)).astype(BF),
            **consts,
        })

    with_bias = bool(np.any(bk) or np.any(bv))
    res = run_bass_kernel_spmd(_get_nc(with_bias), in_maps,
                               core_ids=list(range(8))).results

    out = np.empty((L, NB, E), np.float32)
    for c in range(8):
        b, hh = divmod(c, 2)
        out[:, b, hh * OC:(hh + 1) * OC] = res[c]["out"].astype(np.float32)
    return out


if __name__ == "__main__":
    nc = build_nc()
    print("build OK")



# revision 49
# speedup vs baseline: 1.1175x; 1.1175x over previous
"""Cosformer (linear) attention kernel for 8 TRN2 NeuronCores.

Full (unsharded) inputs in, full output out.  Sharding: 8 cores =
4 batches x 2 head-halves.  Core c handles batch b = c//2 and heads
[hh*8, hh*8+8) where hh = c%2, i.e. embed cols [hh*512, (hh+1)*512).

Per-core math (all shapes per core):
  xT = x[:, b, :].T                        (E=1024, L=2048)  for q/k/v
  k  = relu(x_k @ Wk_s.T + bk_s)           [L, 512]  (L on partitions)
  v  =      x_v @ Wv_s.T + bv_s            [L, 512]
  qT = relu(Wq_s @ x_q.T + bq_s)           [512, L]  (head dims on partitions)
  per head h (64 dims):
    k_ = [k*sin | k*cos]                   [L, 128]
    KV_aug = k_.T @ [v | 1]                [128, 65]   (col 64 = sum_l k_)
  attention (no q duplication): with qts = qT*sin_l, qtc = qT*cos_l
  (per-L-column scaling on DVE, one pass per quarter), and block-diagonal
  KV tensors kvs_bd/kvc_bd [128, oc, 130] pairing head 2oc (rows 0:64,
  cols 0:65) with head 2oc+1 (rows 64:128, cols 65:130):
    po = qts_chunk.T @ kvs_bd[oc] + qtc_chunk.T @ kvc_bd[oc]   (PSUM acc)
    po[:, t*65 : t*65+65] = o_aug of head 2oc+t
    o = o_aug[:, :64] / max(o_aug[:, 64], EPS)
"""

import math
from contextlib import ExitStack

import numpy as np
import ml_dtypes

BF = ml_dtypes.bfloat16

import concourse.bass as bass
import concourse.bacc as bacc_mod
import concourse.mybir as mybir
from concourse.tile import TileContext
from concourse.bass_utils import run_bass_kernel_spmd

L = 2048            # sequence length
NB = 4              # batch
E = 1024            # embed dim
D = 64              # head dim
HC = 8              # heads per core
OC = HC * D         # 512 embed cols per core
P = 128
KC = E // P         # 8 contraction chunks over E
LC = L // P         # 16 L chunks of 128
NLC = L // 512      # 4 L chunks of 512
OCC = OC // P       # 4 q-proj output chunks
EPS = 1e-4

F32 = mybir.dt.float32
F32R = mybir.dt.float32r
BF16 = mybir.dt.bfloat16
AF = mybir.ActivationFunctionType

BD = 2 * (D + 1)    # block-diag kv free width (two heads' aug columns)


def build_nc(with_bias=True):
    nc = bacc_mod.Bacc()

    # x/w come in pre-swizzled by the host so every DMA slice is contiguous
    # per partition (big descriptors -> full DMA-queue bandwidth):
    #   x*: [P, NLC, KC, 512]  (group-major),  w*: [P, KC, OC]
    xq = nc.declare_dram_parameter("xq", [P, NLC * KC * 512], BF16,
                                   isOutput=False)
    xk = nc.declare_dram_parameter("xk", [P, NLC * KC * 512], BF16,
                                   isOutput=False)
    xv = nc.declare_dram_parameter("xv", [P, NLC * KC * 512], BF16,
                                   isOutput=False)
    wq = nc.declare_dram_parameter("wq", [P, KC * OC], BF16, isOutput=False)
    wk = nc.declare_dram_parameter("wk", [P, KC * OC], BF16, isOutput=False)
    wv = nc.declare_dram_parameter("wv", [P, KC * OC], BF16, isOutput=False)
    bqc = nc.declare_dram_parameter("bqc", [P, OCC], F32, isOutput=False)
    bkr = nc.declare_dram_parameter("bkr", [1, OC], BF16, isOutput=False)
    bvr = nc.declare_dram_parameter("bvr", [1, OC], BF16, isOutput=False)
    onesr = nc.declare_dram_parameter("onesr", [1, P], BF16, isOutput=False)
    scs = nc.declare_dram_parameter("scs", [P, L], BF16, isOutput=False)
    scc = nc.declare_dram_parameter("scc", [P, L], BF16, isOutput=False)
    # sin and cos per-lc columns packed in one tensor: one dma_start,
    # 128B descriptors instead of 2x64B
    sccol = nc.declare_dram_parameter("sccol", [P, 2 * LC], F32,
                                      isOutput=False)
    outd = nc.declare_dram_parameter("out", [L, OC], BF16, isOutput=True)

    xq_r = xq.rearrange("p (g kc l) -> p g kc l", g=NLC, kc=KC)
    xk_r = xk.rearrange("p (g kc l) -> p g kc l", g=NLC, kc=KC)
    xv_r = xv.rearrange("p (g kc l) -> p g kc l", g=NLC, kc=KC)
    wk_r = wk.rearrange("p (kc o) -> p kc o", kc=KC)
    wv_r = wv.rearrange("p (kc o) -> p kc o", kc=KC)
    wq_r = wq.rearrange("p (kc o) -> p kc o", kc=KC)
    out_r = outd.rearrange("(lc p) o -> lc p o", p=P)

    with TileContext(nc) as tc, ExitStack() as ctx:
        const = ctx.enter_context(tc.tile_pool(name="const", bufs=1))
        persist = ctx.enter_context(tc.tile_pool(name="persist", bufs=1))
        xqp = ctx.enter_context(tc.tile_pool(name="xqp", bufs=1))


        wk_t = const.tile([P, KC, OC], BF16)
        wv_t = const.tile([P, KC, OC], BF16)
        wq_t = const.tile([P, KC, OC], BF16)
        bq_t = const.tile([P, OCC], F32)
        bk_t = const.tile([1, OC], BF16)
        bv_t = const.tile([1, OC], BF16)
        sc_col = const.tile([P, 2, LC], F32)
        ones_t = const.tile([1, P], BF16)

        scs_t = persist.tile([P, L], BF16)   # sin_l broadcast to all rows
        scc_t = persist.tile([P, L], BF16)   # cos_l broadcast to all rows
        # per-quarter tiles (separate tiles keep dep-tracking fine-grained).
        # qt_sb holds raw relu(q); qts_sb = qt*sin; qt_sb is then overwritten
        # in place with qt*cos.
        qt_sb = [persist.tile([P, OCC, 512], BF16, name=f"qt{n}")
                 for n in range(NLC)]
        qts_sb = [persist.tile([P, OCC, 512], BF16, name=f"qts{n}")
                  for n in range(NLC)]
        kv_sb = persist.tile([P, HC, D + 2], BF16)   # per-head KV_aug
        kvs_bd = persist.tile([P, OCC, BD], BF16)    # block-diag sin-KV
        kvc_bd = persist.tile([P, OCC, BD], BF16)    # block-diag cos-KV

        # ---------------- phase 1: k/v projections + KV accumulation -------
        with ExitStack() as p1:
            xkp = p1.enter_context(tc.tile_pool(name="xkp", bufs=1))
            xvp = p1.enter_context(tc.tile_pool(name="xvp", bufs=1))
            warmp = p1.enter_context(tc.tile_pool(name="warmp", bufs=1))
            kscp = p1.enter_context(tc.tile_pool(name="kscp", bufs=6))
            vap = p1.enter_context(tc.tile_pool(name="vap", bufs=3))
            projp = p1.enter_context(tc.tile_pool(name="projp", bufs=4,
                                                  space="PSUM"))
            kvp = p1.enter_context(tc.tile_pool(name="kvp", bufs=1,
                                                space="PSUM"))

            kv_ps = [
                kvp.tile([P, 4, D + 2], F32, name="kv_ps0"),
                kvp.tile([P, 4, D + 2], F32, name="kv_ps1"),
            ]

            # HAM warm-up: keep PE busy during the initial DMA ramp so the
            # clock gate opens before the first real matmuls.  Results are
            # discarded (kv_ps0 is reset by the real chain's start=True).
            warm_t = warmp.tile([P, 2 * P], BF16, name="warm_t")
            nc.vector.memset(warm_t[:, :], 0.0)
            for w in range(44):
                nc.tensor.matmul(kv_ps[0][:, 0:2, :], warm_t[:, 0:P],
                                 warm_t[:, 0:2 * (D + 2)],
                                 start=True, stop=True)

            # ---- intro (lc 0-3): kc-major so compute starts as soon as the
            # first (wk chunk, xk chunk) pair lands.  DMA issue order IS the
            # HWDGE service order per queue: interleave per-kc pairs.
            xk_t0 = xkp.tile([P, KC, 512], BF16, tag="xk_g0", name="xk_t0")
            xv_t0 = xvp.tile([P, KC, 512], BF16, tag="xv_g0", name="xv_t0")
            # DMA engine slots cost ~constant time per DESCRIPTOR (one per
            # partition row), so per-partition-contiguous size is king:
            # 8KB descriptors move ~3x the bytes/slot of 1KB ones.  Issue
            # each intro tensor as a small head chunk (first kc, so the
            # first matmul fires ASAP) + one big tail chunk (7 contiguous
            # kc = 7KB descriptors).  Weights ride the sync ring, x the
            # scalar ring, so both streams ramp together.
            intro_chunks = [(0, 1), (1, 4), (4, 8)]
            for c0, c1 in intro_chunks:
                nc.sync.dma_start(out=wk_t[:, c0:c1, :], in_=wk_r[:, c0:c1, :])
                nc.scalar.dma_start(out=xk_t0[:, c0:c1, :],
                                    in_=xk_r[:, 0, c0:c1, :])
            nc.sync.dma_start(out=sc_col, in_=sccol[:, :])
            if with_bias:
                nc.sync.dma_start(out=bk_t, in_=bkr[:, :])
                nc.sync.dma_start(out=bv_t, in_=bvr[:, :])
                nc.sync.dma_start(out=ones_t, in_=onesr[:, :])
            for c0, c1 in intro_chunks:
                nc.sync.dma_start(out=wv_t[:, c0:c1, :], in_=wv_r[:, c0:c1, :])
                nc.scalar.dma_start(out=xv_t0[:, c0:c1, :],
                                    in_=xv_r[:, 0, c0:c1, :])
            # group-1 prefetch (lands while the intro computes); groups 2/3
            # are issued inside the steady loop (ping-pong buffers)
            xk_ts = {0: xk_t0}
            xv_ts = {0: xv_t0}

            def prefetch_x(g):
                xk_tg = xkp.tile([P, KC, 512], BF16, tag=f"xk_g{g % 2}",
                                 name="xk_tg")
                xv_tg = xvp.tile([P, KC, 512], BF16, tag=f"xv_g{g % 2}",
                                 name="xv_tg")
                nc.scalar.dma_start(out=xk_tg, in_=xk_r[:, g])
                nc.sync.dma_start(out=xv_tg, in_=xv_r[:, g])
                xk_ts[g] = xk_tg
                xv_ts[g] = xv_tg

            prefetch_x(1)

            ksc_ts = {}
            va_ts = {}

            def proj_block(tag, x_t, w_t, b_t):
                """kc-major 4-lc projection block; returns 4 psum tiles."""
                p_ts = [projp.tile([P, OC], F32, tag="proj", name=f"p_{tag}{i}")
                        for i in range(4)]
                for kc in range(KC):
                    for i in range(4):
                        nc.tensor.matmul(p_ts[i][:, :],
                                         x_t[:, kc, i * P:(i + 1) * P],
                                         w_t[:, kc, :],
                                         start=(kc == 0),
                                         stop=(not with_bias and kc == KC - 1))
                if with_bias:
                    for i in range(4):
                        nc.tensor.matmul(p_ts[i][:, :], ones_t[:, :], b_t[:, :],
                                         start=False, stop=True)
                return p_ts

            def make_ksc(lc, pk_t):
                # k_sc[p,h,0,:] = relu(k)*sin_l ; k_sc[p,h,1,:] = relu(k)*cos_l
                # (sin/cos >= 0 on (0, pi/2], so relu(k*s) == relu(k)*s)
                ksc_t = kscp.tile([P, HC, 2, D], BF16, tag="ksc", name="ksc_t")
                pk_v = pk_t.rearrange("p (h d) -> p h d", d=D)
                nc.scalar.activation(ksc_t[:, :, 0, :], pk_v, AF.Relu,
                                     scale=sc_col[:, 0, lc:lc + 1])
                nc.scalar.activation(ksc_t[:, :, 1, :], pk_v, AF.Relu,
                                     scale=sc_col[:, 1, lc:lc + 1])
                ksc_ts[lc] = ksc_t

            def make_va(lc, pv_t):
                va_t = vap.tile([P, HC, D + 2], BF16, tag="va", name="va_t")
                pv_v = pv_t.rearrange("p (h d) -> p h d", d=D)
                nc.scalar.activation(va_t[:, :, D:D + 2], pv_v[:, :, 0:2],
                                     AF.Copy, bias=1.0, scale=0.0)
                nc.vector.tensor_copy(va_t[:, :, 0:D], pv_v)
                va_ts[lc] = va_t

            def kv_acc(lc):
                # KV_aug accumulation: 4 heads share one PSUM bank; only the
                # very first matmul into each bank uses start=True (clears
                # has_written bank-wide), everything else start=False so the
                # per-element has_written bits do the right thing.
                ksc_t, va_t = ksc_ts.pop(lc), va_ts.pop(lc)
                for h in range(HC):
                    nc.tensor.matmul(
                        kv_ps[h // 4][:, h % 4, :],
                        ksc_t[:, h, :, :],
                        va_t[:, h, :],
                        start=(lc == 0 and h % 4 == 0),
                        stop=(lc == LC - 1 and h % 4 == 3),
                    )

            pk_ts = proj_block("k", xk_t0, wk_t, bk_t)
            for lc in range(4):
                make_ksc(lc, pk_ts[lc])
            pv_ts = proj_block("v", xv_t0, wv_t, bv_t)
            for lc in range(4):
                make_va(lc, pv_ts[lc])
                kv_acc(lc)

            # ---- steady (lc 4-15): lc-major
            xq_ts = []
            for lc in range(4, LC):
                g = lc // 4
                if lc == 4:
                    prefetch_x(2)
                elif lc == 8:
                    prefetch_x(3)
                    # q-phase loads sit AFTER prefetch_x(3) in the ring
                    # FIFOs, so they stream through the otherwise-idle back
                    # half of phase 1 instead of competing with the k/v
                    # steady prefetches.
                    nc.scalar.dma_start(out=wq_t, in_=wq_r)
                    for n in range(2):
                        xq_t = xqp.tile([P, KC, 512], BF16, tag=f"xq{n}",
                                        name="xq_t")
                        eng = nc.sync if n % 2 == 0 else nc.scalar
                        eng.dma_start(out=xq_t, in_=xq_r[:, n])
                        xq_ts.append(xq_t)
                    nc.sync.dma_start(out=scs_t, in_=scs[:, :])
                    nc.scalar.dma_start(out=scc_t, in_=scc[:, :])
                elif lc == 10:
                    for n in range(2, NLC):
                        xq_t = xqp.tile([P, KC, 512], BF16, tag=f"xq{n}",
                                        name="xq_t")
                        eng = nc.sync if n % 2 == 0 else nc.scalar
                        eng.dma_start(out=xq_t, in_=xq_r[:, n])
                        xq_ts.append(xq_t)
                    nc.sync.dma_start(out=bq_t, in_=bqc[:, :])
                j4 = (lc % 4) * P
                xk_t = xk_ts[g][:, :, j4:j4 + P]
                xv_t = xv_ts[g][:, :, j4:j4 + P]

                pk_t = projp.tile([P, OC], F32, tag="proj", name="pk_t")
                for kc in range(KC):
                    nc.tensor.matmul(pk_t[:, :], xk_t[:, kc, :], wk_t[:, kc, :],
                                     start=(kc == 0),
                                     stop=(not with_bias and kc == KC - 1))
                if with_bias:
                    nc.tensor.matmul(pk_t[:, :], ones_t[:, :], bk_t[:, :],
                                     start=False, stop=True)
                make_ksc(lc, pk_t)
                # kv_acc lags one lc: lc-1's ksc/va (ACT+DVE latency off the
                # k-proj psum) completes under THIS lc's k matmuls, so the
                # PE never waits on the epilogue chain -- in particular not
                # at the phase-1 -> phase-2 boundary.
                if lc > 4:
                    kv_acc(lc - 1)

                pv_t = projp.tile([P, OC], F32, tag="proj", name="pv_t")
                for kc in range(KC):
                    nc.tensor.matmul(pv_t[:, :], xv_t[:, kc, :], wv_t[:, kc, :],
                                     start=(kc == 0),
                                     stop=(not with_bias and kc == KC - 1))
                if with_bias:
                    nc.tensor.matmul(pv_t[:, :], ones_t[:, :], bv_t[:, :],
                                     start=False, stop=True)
                make_va(lc, pv_t)

            def qproj_quarter(n, hook=None):
                # per-oc: matmuls, relu evict, then the sin/cos scaling muls
                # IMMEDIATELY (per-oc on DVE): each mul's relu dependency
                # resolves partway through the quarter, so the muls drain
                # during the quarter's own matmul window and never
                # head-of-line-block a later epilogue in the DVE FIFO.
                qt_n, qts_n = qt_sb[n], qts_sb[n]
                r = slice(n * 512, (n + 1) * 512)
                for oc in range(OCC):
                    pq_t = pqp.tile([P, 512], F32, tag="pq", name="pq_t")
                    for kc in range(KC):
                        nc.tensor.matmul(
                            pq_t[:, :],
                            wq_t[:, kc, oc * P:(oc + 1) * P],
                            xq_ts[n][:, kc, :],
                            start=(kc == 0), stop=(kc == KC - 1))
                    nc.scalar.activation(qt_n[:, oc, :], pq_t[:, :],
                                         AF.Relu, bias=bq_t[:, oc:oc + 1])
                    nc.vector.tensor_tensor(qts_n[:, oc, :], qt_n[:, oc, :],
                                            scs_t[:, r],
                                            mybir.AluOpType.mult)
                    nc.vector.tensor_tensor(qt_n[:, oc, :], qt_n[:, oc, :],
                                            scc_t[:, r],
                                            mybir.AluOpType.mult)
                    if oc == 0 and hook is not None:
                        hook()

            kv_acc(LC - 1)

            # evict KV accumulators to SBUF on ACT
            nc.scalar.activation(kv_sb[:, 0:4, :], kv_ps[0][:, :, :], AF.Copy)
            nc.scalar.activation(kv_sb[:, 4:8, :], kv_ps[1][:, :, :], AF.Copy)

        # ---- block-diagonal KV build (phase boundary; rings are idle) ----
        # kvs_bd[:, oc]: rows 0:64 cols 0:65 = KV_sin of head 2oc,
        #                rows 64:128 cols 65:130 = KV_sin of head 2oc+1.
        # kvc_bd likewise with the cos halves.  Off-blocks must be ZERO
        # (they are accumulated over by the paired matmul).
        nc.vector.memset(kvs_bd[:, :, :], 0.0)
        nc.vector.memset(kvc_bd[:, :, :], 0.0)
        kvv = kv_sb.rearrange("p (o t) m -> p o t m", t=2)
        DA = D + 1
        # partition-preserving halves on DVE
        nc.vector.tensor_copy(kvs_bd[0:D, :, 0:DA], kvv[0:D, :, 0, 0:DA])
        nc.vector.tensor_copy(kvc_bd[D:P, :, DA:BD], kvv[D:P, :, 1, 0:DA])
        # partition-crossing halves via SBUF->SBUF DMA
        nc.sync.dma_start(out=kvs_bd[D:P, :, DA:BD], in_=kvv[0:D, :, 1, 0:DA])
        nc.scalar.dma_start(out=kvc_bd[0:D, :, 0:DA], in_=kvv[D:P, :, 0, 0:DA])

        # ---------------- phase 2: q projection + attention ----------------
        with ExitStack() as p2:
            pqp = p2.enter_context(tc.tile_pool(name="pqp", bufs=2,
                                                space="PSUM"))
            pop = p2.enter_context(tc.tile_pool(name="pop", bufs=3,
                                                space="PSUM"))
            osbp = p2.enter_context(tc.tile_pool(name="osbp", bufs=8))
            zp = p2.enter_context(tc.tile_pool(name="zp", bufs=8))

            def attn_lc(lc):
                # per oc-pair: two accumulating matmuls (sin- and cos-half)
                # into one PSUM region; col t*65+64 is head 2oc+t's
                # denominator.  po tiles span exactly 2 banks (2 oc each).
                # All 8 matmuls are emitted before the epilogues so the PE
                # stream never sits behind an epilogue dependency; one
                # half's final scale runs on ACT to balance DVE.
                n = lc // 4
                j4 = (lc % 4) * P
                o_t = osbp.tile([P, OC], BF16, tag="osb", name="o_t")
                ov = o_t.rearrange("p (o t d) -> p o t d", o=OCC, d=D)
                for half in range(2):
                    po_t = pop.tile([P, 2, 512], F32, tag="po", name="po_t")
                    for i in range(2):
                        oc = half * 2 + i
                        nc.tensor.matmul(po_t[:, i, 0:BD],
                                         qts_sb[n][:, oc, j4:j4 + P],
                                         kvs_bd[:, oc, :],
                                         start=True, stop=False)
                        nc.tensor.matmul(po_t[:, i, 0:BD],
                                         qt_sb[n][:, oc, j4:j4 + P],
                                         kvc_bd[:, oc, :],
                                         start=False, stop=True)
                    pov = po_t[:, :, 0:BD].rearrange(
                        "p i (t m) -> p i t m", t=2)
                    # the eps clip never binds: den is a sum of 2048
                    # nonnegative relu products, orders of magnitude > eps
                    zr_t = zp.tile([P, 2, 2], F32, tag="zr", name="zr_t")
                    nc.vector.reciprocal(zr_t[:, :, :], pov[:, :, :, D])
                    zb = zr_t[:, :, :].unsqueeze(3).broadcast_to(
                        (P, 2, 2, D))
                    nc.vector.tensor_tensor(ov[:, 2 * half:2 * half + 2],
                                            pov[:, :, :, 0:D], zb,
                                            mybir.AluOpType.mult)
                eng = nc.sync if lc % 2 == 0 else nc.scalar
                eng.dma_start(out=out_r[lc], in_=o_t[:, :])

            # software-pipelined: attention block b launches one quarter
            # after q-proj quarter b; its only dependencies are qts/qtc
            # (DVE, right after quarter b's relu evicts) and the kv_bd
            # tiles (built at the phase boundary).  No DMA on the critical
            # path, so only block 3 trails the last q-proj.
            qproj_quarter(0)
            qproj_quarter(1)
            for lc in range(0, 4):
                attn_lc(lc)
            qproj_quarter(2)
            for lc in range(4, 8):
                attn_lc(lc)
            qproj_quarter(3)
            for lc in range(8, 16):
                attn_lc(lc)

    nc.compile()
    return nc


_NC = {}


def _get_nc(with_bias=True):
    if with_bias not in _NC:
        _NC[with_bias] = build_nc(with_bias)
    return _NC[with_bias]


def _host_constants():
    idx = (math.pi / 2.0) * (np.arange(L, dtype=np.float64) + 1.0) / float(L)
    sinv = np.sin(idx).astype(np.float32)
    cosv = np.cos(idx).astype(np.float32)
    return {
        "scs": np.ascontiguousarray(
            np.broadcast_to(sinv[None, :], (P, L))).astype(BF),
        "scc": np.ascontiguousarray(
            np.broadcast_to(cosv[None, :], (P, L))).astype(BF),
        "sccol": np.ascontiguousarray(np.stack(
            [sinv.reshape(LC, P).T, cosv.reshape(LC, P).T],
            axis=1).reshape(P, 2 * LC)),
        "onesr": np.ones((1, P), BF),
    }


def _swizzle_x(xT):
    """[E, L] -> [P, NLC*KC*512]: x[p, g, kc, j] = xT[kc*P+p, g*512+j]."""
    v = xT.reshape(KC, P, NLC, 512).transpose(1, 2, 0, 3)
    return np.ascontiguousarray(v).reshape(P, NLC * KC * 512)


def _swizzle_w(wT):
    """[E, OC] -> [P, KC*OC]: w[p, kc, o] = wT[kc*P+p, o]."""
    v = wT.reshape(KC, P, OC).transpose(1, 0, 2)
    return np.ascontiguousarray(v).reshape(P, KC * OC)


def kernel(query, key, value, Wq, bq, Wk, bk, Wv, bv):
    query = np.asarray(query, np.float32)
    key = np.asarray(key, np.float32)
    value = np.asarray(value, np.float32)
    Wq = np.asarray(Wq, np.float32)
    Wk = np.asarray(Wk, np.float32)
    Wv = np.asarray(Wv, np.float32)
    bq = np.asarray(bq, np.float32)
    bk = np.asarray(bk, np.float32)
    bv = np.asarray(bv, np.float32)

    consts = _host_constants()
    in_maps = []
    for c in range(8):
        b, hh = divmod(c, 2)
        sl = slice(hh * OC, (hh + 1) * OC)
        in_maps.append({
            "xq": _swizzle_x(query[:, b, :].T).astype(BF),
            "xk": _swizzle_x(key[:, b, :].T).astype(BF),
            "xv": _swizzle_x(value[:, b, :].T).astype(BF),
            "wq": _swizzle_w(Wq[sl, :].T).astype(BF),
            "wk": _swizzle_w(Wk[sl, :].T).astype(BF),
            "wv": _swizzle_w(Wv[sl, :].T).astype(BF),
            "bqc": np.ascontiguousarray(bq[sl].reshape(OCC, P).T),
            "bkr": np.ascontiguousarray(bk[sl].reshape(1, OC)).astype(BF),
            "bvr": np.ascontiguousarray(bv[sl].reshape(1, OC)).astype(BF),
            **consts,
        })

    with_bias = bool(np.any(bk) or np.any(bv))
    res = run_bass_kernel_spmd(_get_nc(with_bias), in_maps,
                               core_ids=list(range(8))).results

    out = np.empty((L, NB, E), np.float32)
    for c in range(8):
        b, hh = divmod(c, 2)
        out[:, b, hh * OC:(hh + 1) * OC] = res[c]["out"].astype(np.float32)
    return out


if __name__ == "__main__":
    nc = build_nc()
    print("build OK")

